# revision 79
# baseline (speedup 1.0000x reference)
"""GNN message-passing layer on 8 trn2 NeuronCores.

Math: messages = relu(x_src@W1 + x_tgt@W2 + b); agg = mean over target;
out = relu(concat(x, agg) @ W_upd + bu).

Plan (target-sharded; host does index work, the A-row gather, and constant
prep only):
  L1 (device): per-core node shard -> A^T=(x@W1)^T and negB'^T=-(x@W2+b)^T
      in one K=66 matmul (ones-row folds the bias), bf16.
  Host: sorts edges by target, nodes by in-degree descending; builds the
      "plane" stream: plane j = the j-th edge slot of every node with
      degree > j, so every slice the device touches is packed.  Gathers A
      rows into that stream (pure data movement; ~2/3 bf16 for DVE 2x
      throughput, 1/3 fp8 to cut DMA bytes).
  L2 (device): Y = max(G, -B') on DVE (relu(G+B') = max(G,-B') + B'; the
      +B' is folded into the PSUM init cnt_corr*B', which also cancels pad
      slots exactly), then PE identity-matmul injection into a PSUM f32
      accumulator (1024-node-column chunks).  agg = acc * (1/count) (ACT
      copy + DVE mult), then the update MLP + relu runs per chunk,
      software-pipelined one chunk behind.

The plane schedule is baked into the NEFF at kernel() time from the actual
edge_index (one SPMD schedule = max across cores; pad slots absorb skew).
"""

import numpy as np
import ml_dtypes

import concourse.bacc as bacc
import concourse.mybir as mybir
import concourse.tile as tile
from concourse.bass_utils import run_bass_kernel_spmd
from concourse.masks import make_identity

N_NODES = 100000
N_EDGES = 1600000
CORES = 8
NPC = N_NODES // CORES          # 12500 nodes per core
NHALF = 6272                    # per-half columns (2*6272 = 12544 >= 12500)
NPAD1 = 12800                   # L1 padded cols (25 x 512)
CHUNKN = 1024                   # node-columns per PSUM accumulation chunk
SEG = 512                       # segment width
DTILE = 8192                    # stream DMA tile width (elements)
# slab assignment pattern: (dtype, path) per slab ordinal mod len(pattern).
# dtype: 0=bf16, 1=fp8.  path: 0 = max(G,-B') on DVE; 1 = relu via PE-inject
# + ACT (PSUM staging), which frees DVE at the cost of PE/ACT work.
SLAB_PATTERN = ((1, 0), (0, 0), (0, 0), (0, 0), (0, 0))
HAS_RELU_PATH = any(p == 1 for _, p in SLAB_PATTERN)

bf16 = mybir.dt.bfloat16
f32 = mybir.dt.float32
fp8 = mybir.dt.float8e4
BF = ml_dtypes.bfloat16
F8 = ml_dtypes.float8_e4m3
# pad value: max(PAD, -B') must equal -B' for any B' (|B'| < 4), and stay
# finite in fp8e4m3 (max 240)
PAD_G = np.float32(-240.0)

_cache = {}


def _plane_schedule(K):
    """Shared host/device schedule with two streams (0=bf16, 1=fp8).

    Returns (chunks, segs, SH) with SH = [SH_bf16, SH_fp8].
    chunks: list of (a, b, slabs, parts) per node-column chunk.
      slabs: (dt, off) - one full-width pair slab ([s x (planeA ws|planeB ws)],
        ws = min(SEG, b-a)) at offset off of stream dt.
      parts: (s, ws, dt, off) - partial-pair segment pair at offset off
        (planeA at off, planeB at off+ws), covering columns [a+s, a+s+ws).
    segs: flat (plane_j, col0, ws, dt, off) for the host gather.
    Slabs never straddle DTILE boundaries; alignment gaps are pad slots no
    compute op reads."""
    npair = len(K) // 2
    chunks = []
    segs = []
    cur = [0, 0]
    sli = 0  # global slab ordinal for dtype assignment
    a = 0

    def align(dt, need):
        if cur[dt] // DTILE != (cur[dt] + need - 1) // DTILE:
            cur[dt] = ((cur[dt] // DTILE) + 1) * DTILE

    while a < NHALF:
        b = min(a + CHUNKN, NHALF)
        w_ch = b - a
        ws_f = min(SEG, w_ch)
        n_s = (w_ch + ws_f - 1) // ws_f
        slab = 2 * w_ch
        full = [p for p in range(npair) if K[2 * p] >= b]
        part = [p for p in range(npair) if a < K[2 * p] < b]
        parts = []
        for p in part:
            w = K[2 * p] - a
            s = 0
            while s < w:
                ws = min(SEG, w - s)
                align(1, 2 * ws)
                off = cur[1]
                parts.append((s, ws, 1, off))
                segs.append((2 * p, a + s, ws, 1, off, 0))
                segs.append((2 * p + 1, a + s, ws, 1, off + ws, 0))
                cur[1] += 2 * ws
                s += ws
        slabs = []
        for p in full:
            dt, path = SLAB_PATTERN[sli % len(SLAB_PATTERN)]
            sli += 1
            align(dt, slab)
            off = cur[dt]
            for si in range(n_s):
                o = off + si * 2 * ws_f
                segs.append((2 * p, a + si * ws_f, ws_f, dt, o, path))
                segs.append((2 * p + 1, a + si * ws_f, ws_f, dt, o + ws_f,
                             path))
            slabs.append((dt, off, path))
            cur[dt] += slab
        chunks.append((a, b, slabs, parts))
        a = b
    SH = [((c + DTILE - 1) // DTILE) * DTILE for c in cur]
    return chunks, segs, SH


def _build_l1():
    nc = bacc.Bacc("TRN2", debug=False, num_devices=CORES)
    xt65 = nc.dram_tensor("xt65", [66, NPAD1], bf16, kind="ExternalInput")
    wab = nc.dram_tensor("wab", [66, 128], bf16, kind="ExternalInput")
    ab = nc.dram_tensor("ab", [128, NPAD1], bf16, kind="ExternalOutput")

    QW = 2560  # 5 tiles of 512 per DMA piece
    with tile.TileContext(nc) as tc:
        with (
            tc.tile_pool(name="big", bufs=1) as big,
            tc.tile_pool(name="psum", bufs=4, space="PSUM") as psum,
        ):
            wt = big.tile([66, 128], bf16)
            xt = big.tile([66, NPAD1], bf16)
            abt = big.tile([128, NPAD1], bf16)
            nc.sync.dma_start(out=wt[:], in_=wab[:, :])
            for q in range(NPAD1 // QW):
                qs = slice(q * QW, (q + 1) * QW)
                nc.sync.dma_start(out=xt[:, qs], in_=xt65[:, qs])
            for c in range(NPAD1 // 512):
                sl = slice(c * 512, (c + 1) * 512)
                pt = psum.tile([128, 512], f32)
                nc.tensor.matmul(out=pt[:], lhsT=wt[:], rhs=xt[:, sl],
                                 start=True, stop=True)
                if c % 2 == 0:
                    nc.vector.tensor_copy(out=abt[:, sl], in_=pt[:])
                else:
                    nc.scalar.activation(
                        out=abt[:, sl], in_=pt[:],
                        func=mybir.ActivationFunctionType.Copy)
                if (c + 1) % 5 == 0:
                    qs = slice((c + 1) * 512 - QW, (c + 1) * 512)
                    nc.sync.dma_start(out=ab[:, qs], in_=abt[:, qs])
    nc.compile()
    return nc


def _build_l2(K):
    chunks, _segs, SH = _plane_schedule(K)
    nc = bacc.Bacc("TRN2", debug=False, num_devices=CORES)
    g16 = nc.dram_tensor("g16", [128, SH[0]], bf16, kind="ExternalInput")
    g8 = nc.dram_tensor("g8", [128, SH[1]], fp8, kind="ExternalInput")
    nb = nc.dram_tensor("nb", [128, NHALF], bf16, kind="ExternalInput")
    ic = nc.dram_tensor("ic", [128, NHALF], bf16, kind="ExternalInput")
    ini = nc.dram_tensor("ini", [128, NHALF], bf16, kind="ExternalInput")
    xu = nc.dram_tensor("xu", [128, NHALF], bf16, kind="ExternalInput")
    wu = nc.dram_tensor("wu", [128, 128], bf16, kind="ExternalInput")
    bu = nc.dram_tensor("bu", [64, 1], f32, kind="ExternalInput")
    upd = nc.dram_tensor("upd", [128, NHALF], bf16, kind="ExternalOutput")

    amax = mybir.AluOpType.max
    mult = mybir.AluOpType.mult
    gdram = (g16, g8)
    gdt = (bf16, fp8)
    ntile = (SH[0] // DTILE, SH[1] // DTILE)

    with tile.TileContext(nc) as tc:
        with (
            tc.tile_pool(name="persist", bufs=1) as per,
            tc.tile_pool(name="st16", bufs=3) as st16p,
            tc.tile_pool(name="st8", bufs=3) as st8p,
            tc.tile_pool(name="ybuf", bufs=4) as ybuf,
            tc.tile_pool(name="ypart", bufs=6) as ypart,
            tc.tile_pool(name="abuf", bufs=3) as abuf,
            tc.tile_pool(name="rbuf", bufs=4) as rbuf,
            tc.tile_pool(name="obuf", bufs=3) as obuf,
            tc.tile_pool(name="acc", bufs=2, space="PSUM") as accp,
            tc.tile_pool(name="upsum", bufs=2, space="PSUM") as upsum,
            tc.tile_pool(name="stage", bufs=2, space="PSUM") as stagep,
        ):
            nb_t = per.tile([128, NHALF], bf16)
            ic_t = per.tile([128, NHALF], bf16)
            ini_t = per.tile([128, NHALF], bf16)
            xu_t = per.tile([128, NHALF], bf16)
            wu_t = per.tile([128, 128], bf16)
            bu_t = per.tile([64, 1], f32)
            agg_t = per.tile([128, NHALF], bf16)
            if HAS_RELU_PATH:
                nbp_t = per.tile([128, NHALF], bf16)
            ident = per.tile([128, 128], bf16)
            ident8 = per.tile([128, 128], fp8)
            nc.scalar.dma_start(out=wu_t[:], in_=wu[:, :])
            nc.scalar.dma_start(out=bu_t[:], in_=bu[:, :])
            make_identity(nc, ident[:])
            if HAS_RELU_PATH:
                make_identity(nc, ident8[:])

            st_tiles = [{}, {}]
            stp = (st16p, st8p)

            def stile(dt, i):
                cachebin = st_tiles[dt]
                if i not in cachebin:
                    t = stp[dt].tile([128, DTILE], gdt[dt], tag="st")
                    h = DTILE // 2
                    nc.sync.dma_start(
                        out=t[:, 0:h],
                        in_=gdram[dt][:, i * DTILE:i * DTILE + h])
                    nc.sync.dma_start(
                        out=t[:, h:],
                        in_=gdram[dt][:, i * DTILE + h:(i + 1) * DTILE])
                    cachebin[i] = t
                return cachebin[i]

            def prologue(ci):
                a, b, _s, _p = chunks[ci]
                nc.sync.dma_start(out=nb_t[:, a:b], in_=nb[:, a:b])
                nc.sync.dma_start(out=ini_t[:, a:b], in_=ini[:, a:b])
                if HAS_RELU_PATH:
                    nc.vector.tensor_scalar_mul(out=nbp_t[:, a:b],
                                                in0=nb_t[:, a:b],
                                                scalar1=-1.0)

            def finish(a, b, acc_t):
                # per-tile: SBUF copy of acc (ACT), agg = copy * (1/count)
                # (DVE), update MLP (PE), relu+bias (ACT) into chunk-wide
                # per-half buffers; one out DMA per half per chunk
                och0 = obuf.tile([64, CHUNKN], bf16, tag="ot0")
                och1 = obuf.tile([64, CHUNKN], bf16, tag="ot1")
                och = (och0, och1)
                t0 = a
                while t0 < b:
                    w = min(SEG, b - t0)
                    sl = slice(t0, t0 + w)
                    at = abuf.tile([128, SEG], bf16, tag="at")
                    nc.scalar.activation(
                        out=at[:, 0:w], in_=acc_t[:, t0 - a:t0 - a + w],
                        func=mybir.ActivationFunctionType.Copy)
                    nc.vector.tensor_tensor(out=agg_t[:, sl], in0=at[:, 0:w],
                                            in1=ic_t[:, sl], op=mult)
                    for h in range(2):
                        ps = slice(64 * h, 64 * h + 64)
                        pt = upsum.tile([64, SEG], f32, tag=f"pt{h}")
                        nc.tensor.matmul(out=pt[:, 0:w], lhsT=wu_t[ps, 0:64],
                                         rhs=xu_t[ps, sl],
                                         start=True, stop=False)
                        nc.tensor.matmul(out=pt[:, 0:w],
                                         lhsT=wu_t[ps, 64:128],
                                         rhs=agg_t[ps, sl],
                                         start=False, stop=True)
                        nc.scalar.activation(
                            out=och[h][:, t0 - a:t0 - a + w], in_=pt[:, 0:w],
                            func=mybir.ActivationFunctionType.Relu,
                            bias=bu_t[:])
                    t0 += w
                for h in range(2):
                    ps = slice(64 * h, 64 * h + 64)
                    q = (nc.sync, nc.scalar)[h]
                    q.dma_start(out=upd[ps, a:b], in_=och[h][:, 0:b - a])

            prologue(0)
            prev = None
            for ci, (a, b, slabs, parts) in enumerate(chunks):
                w_ch = b - a
                ws_f = min(SEG, w_ch)
                n_s = (w_ch + ws_f - 1) // ws_f
                slab = 2 * w_ch
                acc_t = accp.tile([128, CHUNKN], f32, tag="acc")
                # ISA: matmul rhs <= [128, 512]
                for s0 in range(0, w_ch, SEG):
                    w = min(SEG, w_ch - s0)
                    nc.tensor.matmul(out=acc_t[:, s0:s0 + w], lhsT=ident[:],
                                     rhs=ini_t[:, a + s0:a + s0 + w],
                                     start=True, stop=False)
                if ci + 1 < len(chunks):
                    prologue(ci + 1)
                n_inj = 2 * n_s * len(slabs) + 2 * len(parts)
                inj = 0
                pend = None

                def inject(rhs_ap, s0, ws, last):
                    nc.tensor.matmul(out=acc_t[:, s0:s0 + ws], lhsT=ident[:],
                                     rhs=rhs_ap, start=False, stop=last)

                for (s, ws, dt, off) in parts:
                    st = stile(dt, off // DTILE)
                    la = off % DTILE
                    cols = slice(a + s, a + s + ws)
                    yt = ypart.tile([128, 2 * SEG], bf16, tag="yp")
                    nbb = nb_t[:, cols].unsqueeze(1).to_broadcast([128, 2, ws])
                    nc.vector.tensor_tensor(out=yt[:, 0:2 * ws],
                                            in0=st[:, la:la + 2 * ws],
                                            in1=nbb, op=amax)
                    inj += 2
                    inject(yt[:, 0:ws], s, ws, False)
                    inject(yt[:, ws:2 * ws], s, ws, inj == n_inj)
                # -B' pattern matching one slab's [s x (A|B)] layout
                base = nb_t[:, a:b]
                if n_s > 1:
                    v = base.rearrange("p (s c) -> p s c", s=n_s)
                    v = v.unsqueeze(2).to_broadcast([128, n_s, 2, ws_f])
                else:
                    v = base.unsqueeze(1).to_broadcast([128, 2, w_ch])
                for si_, (dt, off, path) in enumerate(slabs):
                    ti = off // DTILE
                    la = off % DTILE
                    st = stile(dt, ti)
                    if ti + 1 < ntile[dt]:
                        stile(dt, ti + 1)  # prefetch
                    if si_ == 2 and prev is not None:
                        # software pipeline: previous chunk's finish behind
                        # this chunk's first slabs
                        finish(*prev)
                        prev = None
                    if path == 1:
                        # relu path: stage g + B' in PSUM (PE), relu on ACT,
                        # re-inject one unit behind so PE never waits on ACT.
                        for si in range(n_s):
                            for pl in range(2):
                                lo = la + si * 2 * ws_f + pl * ws_f
                                cs = slice(a + si * ws_f, a + (si + 1) * ws_f)
                                sg = stagep.tile([128, SEG], f32, tag="sg")
                                nc.tensor.matmul(
                                    out=sg[:, 0:ws_f], lhsT=ident8[:],
                                    rhs=st[:, lo:lo + ws_f],
                                    start=True, stop=False)
                                nc.tensor.matmul(
                                    out=sg[:, 0:ws_f], lhsT=ident[:],
                                    rhs=nbp_t[:, cs], start=False, stop=True)
                                yo = rbuf.tile([128, SEG], bf16, tag="yo")
                                nc.scalar.activation(
                                    out=yo[:, 0:ws_f], in_=sg[:, 0:ws_f],
                                    func=mybir.ActivationFunctionType.Relu)
                                if pend is not None:
                                    inj += 1
                                    inject(pend[0][:, 0:pend[2]], pend[1],
                                           pend[2], inj == n_inj)
                                pend = (yo, si * ws_f, ws_f)
                        continue
                    yt = ybuf.tile([128, 2 * CHUNKN], bf16, tag="yt")
                    nc.vector.tensor_tensor(out=yt[:, 0:slab],
                                            in0=st[:, la:la + slab],
                                            in1=v, op=amax)
                    for si in range(n_s):
                        lo = si * 2 * ws_f
                        inj += 2
                        inject(yt[:, lo:lo + ws_f], si * ws_f, ws_f, False)
                        inject(yt[:, lo + ws_f:lo + 2 * ws_f],
                               si * ws_f, ws_f, inj == n_inj)
                if pend is not None:
                    inj += 1
                    inject(pend[0][:, 0:pend[2]], pend[1], pend[2],
                           inj == n_inj)
                    pend = None
                if prev is not None:
                    finish(*prev)
                # update-phase inputs, consumed by finish() one chunk later
                nc.scalar.dma_start(out=ic_t[:, a:b], in_=ic[:, a:b])
                nc.scalar.dma_start(out=xu_t[:, a:b], in_=xu[:, a:b])
                prev = (a, b, acc_t)
            finish(*prev)
    nc.compile()
    return nc


def kernel(x, edge_index, W_msg, b_msg, W_upd, b_upd):
    x = np.asarray(x, dtype=np.float32)
    src = np.asarray(edge_index[0], dtype=np.int64)
    tgt = np.asarray(edge_index[1], dtype=np.int64)
    W_msg = np.asarray(W_msg, dtype=np.float32)
    b_msg = np.asarray(b_msg, dtype=np.float32)
    W_upd = np.asarray(W_upd, dtype=np.float32)
    b_upd = np.asarray(b_upd, dtype=np.float32)

    # ---------------- L1 ----------------
    if "l1" not in _cache:
        _cache["l1"] = _build_l1()
    wab = np.zeros((66, 128), dtype=np.float32)
    wab[:64, :64] = W_msg[:64]
    wab[:64, 64:] = -W_msg[64:]
    wab[64, 64:] = -b_msg
    wab = wab.astype(BF)
    xb = x.astype(BF)
    in1 = []
    for c in range(CORES):
        xt65 = np.zeros((66, NPAD1), dtype=BF)
        xt65[:64, :NPC] = xb[c * NPC:(c + 1) * NPC].T
        xt65[64, :] = np.float32(1.0)
        in1.append({"xt65": xt65, "wab": wab})
    res1 = run_bass_kernel_spmd(_cache["l1"], in1, list(range(CORES)))
    A_T = np.concatenate([np.asarray(r["ab"])[0:64, :NPC]
                          for r in res1.results], axis=1)
    negB_T = np.concatenate([np.asarray(r["ab"])[64:128, :NPC]
                             for r in res1.results], axis=1)

    # ---------------- host: plane schedule ----------------
    counts = np.bincount(tgt, minlength=N_NODES).astype(np.int64)
    order = np.argsort(tgt, kind="stable")
    cum = np.zeros(N_NODES + 1, dtype=np.int64)
    np.cumsum(counts, out=cum[1:])

    colloc = np.empty((CORES, 2, NHALF), dtype=np.int64)
    colnode = np.empty((CORES, 2, NHALF), dtype=np.int64)
    colcnt = np.zeros((CORES, 2, NHALF), dtype=np.int64)
    for c in range(CORES):
        lo = c * NPC
        cnt_loc = np.zeros(2 * NHALF, dtype=np.int64)
        cnt_loc[:NPC] = counts[lo:lo + NPC]
        rank = np.argsort(-cnt_loc, kind="stable")
        for h in range(2):
            nodes = rank[h::2]
            colloc[c, h] = nodes
            colnode[c, h] = np.minimum(nodes, NPC - 1) + lo
            colcnt[c, h] = cnt_loc[nodes]

    tmax = int(colcnt.max())
    tmax += tmax % 2
    K = np.zeros(tmax, dtype=np.int64)
    for j in range(tmax):
        K[j] = int((colcnt > j).sum(axis=2).max())
    for p in range(tmax // 2):
        K[2 * p + 1] = K[2 * p]
    K = [int(k) for k in K if k > 0]
    if len(K) % 2:
        K.append(K[-1])

    key = ("l2", tuple(K))
    if key not in _cache:
        _cache[key] = _build_l2(K)
    chunks, segs, SH = _plane_schedule(K)

    # cnt_corr: planes covering each column via the max-path only (the
    # relu path needs no correction; its pads contribute exactly zero)
    cnt_corr = np.zeros(NHALF, dtype=np.float32)
    for (_jj, col0, ws, _dt, _off, path) in segs:
        if path == 0:
            cnt_corr[col0:col0 + ws] += 1

    A_np = np.asarray(A_T)
    A8 = A_np.astype(F8)
    nB_np = np.asarray(negB_T)
    xbT = np.ascontiguousarray(xb.T)
    wu = np.zeros((128, 128), dtype=np.float32)
    wu[:64, :64] = W_upd[:64]
    wu[:64, 64:] = W_upd[64:]
    wu[64:] = wu[:64]
    wu = wu.astype(BF)
    bu = b_upd.reshape(64, 1).astype(np.float32)

    in2 = []
    for c in range(CORES):
        G16 = np.full((128, SH[0]), PAD_G, dtype=BF)
        G8 = np.full((128, SH[1]), PAD_G, dtype=F8)
        for h in range(2):
            nodes = colnode[c, h]
            ncnt = colcnt[c, h]
            starts = cum[nodes]
            for dt, Gt, At in ((0, G16, A_np), (1, G8, A8)):
                srcflat = np.full(SH[dt], -1, dtype=np.int64)
                for (jj, col0, ws, sdt, off, _path) in segs:
                    if sdt != dt:
                        continue
                    csl = slice(col0, col0 + ws)
                    valid = ncnt[csl] > jj
                    srcflat[off:off + ws] = np.where(
                        valid, starts[csl] + jj, -1)
                have = srcflat >= 0
                idx = src[order[srcflat[have]]]
                Gh = np.full((64, SH[dt]), PAD_G, dtype=Gt.dtype)
                Gh[:, have] = At[:, idx]
                Gt[64 * h:64 * h + 64] = Gh
        nb2 = np.empty((128, NHALF), dtype=BF)
        ic2 = np.empty((128, NHALF), dtype=BF)
        ini2 = np.empty((128, NHALF), dtype=BF)
        xu2 = np.empty((128, NHALF), dtype=BF)
        for h in range(2):
            r = slice(64 * h, 64 * h + 64)
            nbh = nB_np[:, colnode[c, h]]
            nb2[r] = nbh
            xu2[r] = xbT[:, colnode[c, h]]
            ic2[r] = (1.0 / np.maximum(colcnt[c, h], 1)).astype(BF)[None, :]
            # ini = cnt_corr * B' = (-cnt_corr) * (-B')
            ini2[r] = ((-cnt_corr[None, :]) *
                       nbh.astype(np.float32)).astype(BF)
        in2.append({"g16": G16, "g8": G8, "nb": nb2, "ic": ic2, "ini": ini2,
                    "xu": xu2, "wu": wu, "bu": bu})

    res2 = run_bass_kernel_spmd(_cache[key], in2, list(range(CORES)))

    out = np.empty((N_NODES, 64), dtype=np.float32)
    for c in range(CORES):
        upd = np.asarray(res2.results[c]["upd"]).astype(np.float32)
        lo = c * NPC
        for h in range(2):
            loc = colloc[c, h]
            real = loc < NPC
            vals = upd[64 * h:64 * h + 64, :].T
            out[lo + loc[real]] = vals[real]
    return out
